# revision 20
# baseline (speedup 1.0000x reference)
# Trainium2 Bass kernel for nn_Net_4861902979707
#
# Computation (per sample, B = 4194304):
#   X [B, 3, 3] -> 3 pairwise Euclidean distances d = [d01, d02, d12]
#   h1 = elu(d @ W1.T + b1); h2 = elu(h1 @ W2.T + b2); y = h2 @ W3.T + b3
#
# Strategy: pure data parallel over 8 NeuronCores (batch split). Host does
# layout/dtype only: X is cast to fp16 and each 128xT tile is transposed to
# channel-major [128 partitions, 9 coord planes, T samples] so every on-chip
# op is a fat contiguous instruction.
#
# The per-tile computation is a long cross-engine chain; engines execute
# their streams strictly in order, so naive tile-by-tile emission runs at
# the SUM of engine times (the 140us baseline). This kernel emits a 6-deep
# software pipeline with work spread across all five engines + the DMA
# CCE ALU, so at each step every engine's next instruction depends only on
# previous-step work (or an op emitted earlier in the same step):
#   step i   : DMA in tile i
#   step i+1 : DVE diffs (3 fat subs) + squares planes 0-3;
#              GPSIMD squares planes 4-8
#   step i+2 : DMA-accumulate q12 = sq6+sq7+sq8 (SWDGE accum_op=add);
#              PE q01/q02 (identity diag, PSUM); ACT sqrt(q01,q02)
#   step i+3 : ACT sqrt(q12); PE L1 (bias via ones-plane + diag passes);
#              ACT exp1/relu1 (fat); DVE h1 = min(e1,1) + r1
#   step i+4 : PE L2; ACT exp2 + relu2[k=0]; DVE relu2[k=1]
#   step i+5 : DVE m2 = min(e2,1); PE L3 (split-feed r2/m2 + bias);
#              ACT final copy -> fp16; DMA out
# ELU identity: elu(z)+1 = relu(z) + min(exp(z), 1); the +1 shift is
# absorbed into the next layer's bias on the host (b' = b - W @ 1).
# Everything on chip is fp16 (PSUM fp32), rel err ~1e-3.
import os as _os
import numpy as np

B = 4194304
N_CORES = 8
B_CORE = B // N_CORES          # 524288
P = 128                        # partitions
T = int(_os.environ.get("TSZ", "512"))
TILE = P * T
N_TILES = B_CORE // TILE

SQ_DVE = int(_os.environ.get("SQ_DVE", "6"))   # square planes on DVE
RELU2K1 = _os.environ.get("RELU2K1", "act")    # dve | act
RELU1 = _os.environ.get("RELU1", "act")        # act | dve (fat TS from PSUM)
RELU2 = _os.environ.get("RELU2", "act")        # act | dve
FINAL = _os.environ.get("FINAL", "act")        # act | dve
Q12 = _os.environ.get("Q12", "pe")            # dma | pe
DEPTH = 6
# bench-only: multiply tile count inside one bench_small body to measure
# steady-state vs pipeline-ramp contributions
BENCH_MULT = int(_os.environ.get("BENCH_MULT", "1"))

COMPUTE_DT = "fp16"

_CACHE = {}


def _split_sync_waits(nc, mybir, limit=1):
    """This walrus build rejects instructions carrying more than ~1 sem wait
    ("Too many sync wait commands"). Hoist excess waits onto NoOp carrier
    instructions (same engine, immediately before) — engine program order
    preserves the blocking semantics."""
    n_split = 0
    for f in nc.m.functions:
        for b in f.blocks:
            lst = b.instructions
            out = []
            changed = False
            for inst in lst:
                si = inst.sync_info
                if si is not None and len(si.on_wait) > limit:
                    waits = list(si.on_wait)
                    extra, keep = waits[:-limit], waits[-limit:]
                    for wi, w in enumerate(extra):
                        nop = mybir.InstNoOp(
                            name=f"wsplit-{inst.name}-{wi}")
                        nop.engine = inst.engine
                        nop.sync_info = mybir.SyncInfo(
                            on_wait=[w], on_update=[])
                        out.append(nop)
                        n_split += 1
                    inst.sync_info = type(si)(
                        on_wait=keep, on_update=list(si.on_update))
                    changed = True
                out.append(inst)
            if changed:
                b.instructions = out
    return n_split


# WD diag-matrix indices (each a [128,128] fp16 lhsT)
def _iWD_I():
    return 0
def _iWD_W1(k, j):
    return 1 + 3 * k + j
def _iWD_W2(m, j):
    return 7 + 2 * m + j
def _iWD_B1(k):
    return 11 + k
def _iWD_B2(m):
    return 13 + m
def _iWD_W3(j):
    return 15 + j
N_WD = 17
N_WB = 1  # slot 0: b3a (final-activation bias)


def _build(dt_name=None, reps=1, bench_small=False):
    import concourse.bass as bass
    import concourse.tile as tile
    import concourse.mybir as mybir

    f32 = mybir.dt.float32
    f16 = mybir.dt.float16
    Alu = mybir.AluOpType
    Act = mybir.ActivationFunctionType

    nc = bass.Bass()
    NT = 1 if bench_small else N_TILES
    X = nc.dram_tensor("X", [NT * P, 9 * T], f16, kind="ExternalInput")
    WB = nc.dram_tensor("WB", [N_WB], f32, kind="ExternalInput")
    WD = nc.dram_tensor("WD", [N_WD, P, P], f16, kind="ExternalInput")
    Y = nc.dram_tensor("Y", [NT * P, T], f16, kind="ExternalOutput")

    with tile.TileContext(nc) as tc:
        with (
            tc.tile_pool(name="singles", bufs=1) as singles,
            tc.tile_pool(name="xin", bufs=4) as xin,
            tc.tile_pool(name="diffp", bufs=3) as diffp,
            tc.tile_pool(name="distp", bufs=3) as distp,
            tc.tile_pool(name="elup", bufs=3) as elup,
            tc.tile_pool(name="h1p", bufs=3) as h1p,
            tc.tile_pool(name="outp", bufs=3) as outp,
            tc.tile_pool(name="ps1", bufs=1, space="PSUM") as ps1,
        ):
            wb = singles.tile([P, N_WB], f32)
            nc.gpsimd.dma_start(
                out=wb[:],
                in_=bass.AP(tensor=WB[:].tensor, offset=0,
                            ap=[[0, P], [1, N_WB]]))
            wd = singles.tile([P, N_WD, P], f16)
            nc.sync.dma_start(
                out=wd[:],
                in_=bass.AP(tensor=WD[:].tensor, offset=0,
                            ap=[[P, P], [P * P, N_WD], [1, P]]))
            ones = singles.tile([P, T], f16)
            nc.vector.memset(ones[:], 1.0)
            ones2 = singles.tile([P, 2, T], f16)
            nc.vector.memset(ones2[:], 1.0)

            def diag(i):  # [128,128] lhsT AP
                return wd[:, i, :]

            WIN = DEPTH + 1
            st = [dict() for _ in range(WIN)]

            def S(i):
                return st[i % WIN]

            d0 = SQ_DVE  # DVE squares planes [0, d0); GP squares [d0, 9)

            NT_body = N_TILES * (BENCH_MULT if bench_small else 1)

            def step(s):
                i_dma, i_s1, i_s2 = s, s - 1, s - 2
                i_b1, i_b2, i_b3 = s - 3, s - 4, s - 5

                def live(i):
                    return 0 <= i < NT_body

                # 1. DMA in (tile s)
                if live(i_dma):
                    src = 0 if bench_small else i_dma
                    xt = xin.tile([P, 9, T], f16)
                    nc.sync.dma_start(
                        out=xt[:], in_=X[src * P:(src + 1) * P, :])
                    S(i_dma)["xt"] = xt

                # 2a. ACT sqrt(q12)  [q12 DMA-accumulated last step]
                if live(i_b1) and Q12 == "dma":
                    d = S(i_b1)
                    nc.scalar.activation(
                        d["dist"][:, 2, :], d["q12"], Act.Sqrt)

                # 2. PE L1 (bias plane + diag passes)
                if live(i_b1):
                    d = S(i_b1)
                    z1 = ps1.tile([P, 2, T], f32, tag="z1")
                    for k in range(2):
                        nc.tensor.matmul(
                            z1[:, k, :], diag(_iWD_B1(k)), ones[:],
                            start=True, stop=False)
                        for j in range(3):
                            nc.tensor.matmul(
                                z1[:, k, :], diag(_iWD_W1(k, j)),
                                d["dist"][:, j, :],
                                start=False, stop=(j == 2))
                    d["z1"] = z1

                # 4. DVE m2 = min(e2, 1)  [e2 from last step]
                if live(i_b3):
                    d = S(i_b3)
                    m2 = outp.tile([P, 2, T], f16, tag="m2")
                    nc.vector.tensor_tensor(
                        out=m2[:], in0=d["e2"][:], in1=ones2[:],
                        op=Alu.min)
                    d["m2"] = m2

                # 5-7. DVE diffs + squares; GP squares
                if live(i_s1):
                    d = S(i_s1)
                    xt = d["xt"]
                    diff = diffp.tile([P, 9, T], f16)
                    nc.vector.tensor_sub(
                        diff[:, 6:9, :], xt[:, 3:6, :], xt[:, 6:9, :])
                    nc.gpsimd.tensor_mul(
                        diff[:, 6:9, :], diff[:, 6:9, :], diff[:, 6:9, :])
                    nc.vector.tensor_sub(
                        diff[:, 0:3, :], xt[:, 0:3, :], xt[:, 3:6, :])
                    nc.vector.tensor_sub(
                        diff[:, 3:6, :], xt[:, 0:3, :], xt[:, 6:9, :])
                    nc.vector.tensor_mul(
                        diff[:, 0:6, :], diff[:, 0:6, :], diff[:, 0:6, :])
                    d["diff"] = diff

                # 9-10. ACT exp1 / relu1 (fat, bias already in PSUM)
                if live(i_b1):
                    d = S(i_b1)
                    e1 = elup.tile([P, 2, T], f16, tag="e1")
                    r1 = elup.tile([P, 2, T], f16, tag="r1")
                    nc.scalar.activation(e1[:], d["z1"][:], Act.Exp)
                    if RELU1 == "dve":
                        nc.vector.tensor_scalar(
                            out=r1[:], in0=d["z1"][:], scalar1=0.0,
                            scalar2=None, op0=Alu.max)
                    else:
                        nc.scalar.activation(r1[:], d["z1"][:], Act.Relu)
                    d["e1"], d["r1"] = e1, r1

                # 11. DVE h1 = min(e1,1) + r1
                if live(i_b1):
                    d = S(i_b1)
                    h1 = h1p.tile([P, 2, T], f16, tag="h1")
                    nc.vector.tensor_tensor(
                        out=h1[:], in0=d["e1"][:], in1=ones2[:], op=Alu.min)
                    nc.vector.tensor_add(h1[:], h1[:], d["r1"][:])
                    d["h1"] = h1

                # 12. PE q01/q02 (+ q12 if not on DMA)
                if live(i_s2):
                    d = S(i_s2)
                    diff = d["diff"]
                    if Q12 == "dma":
                        q12 = distp.tile([P, T], f16, tag="q12")
                        nc.vector.memset(q12[:], 0.0)
                        q12rep = bass.AP(
                            tensor=q12[:].tensor, offset=q12[:].offset,
                            ap=[q12[:].ap[0], [0, 3], [1, T]])
                        nc.gpsimd.dma_start(
                            out=q12rep, in_=diff[:, 6:9, :],
                            accum_op=Alu.add)
                        d["q12"] = q12
                    npair = 2 if Q12 == "dma" else 3
                    qp = ps1.tile([P, npair, T], f32, tag="q")
                    for pi in range(npair):
                        for c in range(3):
                            nc.tensor.matmul(
                                qp[:, pi, :], diag(_iWD_I()),
                                diff[:, 3 * pi + c, :],
                                start=(c == 0), stop=(c == 2))
                    d["qp"] = qp

                # 13. PE L2 (bias + diag passes over h1)
                if live(i_b2):
                    d = S(i_b2)
                    z2 = ps1.tile([P, 2, T], f32, tag="z2")
                    for m_ in range(2):
                        nc.tensor.matmul(
                            z2[:, m_, :], diag(_iWD_B2(m_)), ones[:],
                            start=True, stop=False)
                        for j in range(2):
                            nc.tensor.matmul(
                                z2[:, m_, :], diag(_iWD_W2(m_, j)),
                                d["h1"][:, j, :],
                                start=False, stop=(j == 1))
                    d["z2"] = z2

                # 14. PE L3 (split feed: bias + w3*(r2_j, m2_j))
                if live(i_b3):
                    d = S(i_b3)
                    yz = ps1.tile([P, T], f32, tag="yz")
                    for j in range(2):
                        nc.tensor.matmul(
                            yz[:], diag(_iWD_W3(j)), d["r2"][:, j, :],
                            start=(j == 0), stop=False)
                    for j in range(2):
                        nc.tensor.matmul(
                            yz[:], diag(_iWD_W3(j)), d["m2"][:, j, :],
                            start=False, stop=(j == 1))
                    d["yz"] = yz

                # 15. ACT sqrt (pairs summed in PSUM, fat)
                if live(i_s2):
                    d = S(i_s2)
                    dist = distp.tile([P, 3, T], f16, tag="dist")
                    if Q12 == "dma":
                        nc.scalar.activation(
                            dist[:, 0:2, :], d["qp"][:], Act.Sqrt)
                    else:
                        nc.scalar.activation(
                            dist[:, :, :], d["qp"][:], Act.Sqrt)
                    d["dist"] = dist

                # 16-18. elu2: ACT exp2 (fat) + relu2 split ACT/DVE
                if live(i_b2):
                    d = S(i_b2)
                    e2 = elup.tile([P, 2, T], f16, tag="e2")
                    r2 = elup.tile([P, 2, T], f16, tag="r2")
                    nc.scalar.activation(e2[:], d["z2"][:], Act.Exp)
                    if RELU2 == "dve":
                        nc.vector.tensor_scalar(
                            out=r2[:], in0=d["z2"][:], scalar1=0.0,
                            scalar2=None, op0=Alu.max)
                    elif RELU2K1 == "dve":
                        nc.scalar.activation(
                            r2[:, 0, :], d["z2"][:, 0, :], Act.Relu)
                        nc.vector.tensor_scalar(
                            out=r2[:, 1, :], in0=d["z2"][:, 1, :],
                            scalar1=0.0, scalar2=None, op0=Alu.max)
                    else:
                        nc.scalar.activation(r2[:], d["z2"][:], Act.Relu)
                    
                    d["e2"], d["r2"] = e2, r2

                # 19. final: add b3a, downcast -> fp16
                if live(i_b3):
                    d = S(i_b3)
                    yt = outp.tile([P, T], f16, tag="yt")
                    if FINAL == "dve":
                        nc.vector.tensor_scalar(
                            out=yt[:], in0=d["yz"][:], scalar1=wb[:, 0:1],
                            scalar2=None, op0=Alu.add)
                    else:
                        nc.scalar.activation(
                            yt[:], d["yz"][:], Act.Identity,
                            bias=wb[:, 0:1], scale=1.0)
                    d["yt"] = yt

                # 20. DMA out
                if live(i_b3):
                    src = 0 if bench_small else i_b3
                    nc.sync.dma_start(
                        out=Y[src * P:(src + 1) * P, :], in_=S(i_b3)["yt"][:])

            _loop = tc.For_i(0, reps) if reps != 1 else None
            if _loop is not None:
                _loop.__enter__()

            for s in range(NT_body + DEPTH - 1):
                step(s)

            if _loop is not None:
                _loop.__exit__(None, None, None)

    _split_sync_waits(nc, mybir, limit=1)
    return nc


def _pack_weights(W1, b1, W2, b2, W3, b3):
    W1 = np.asarray(W1, np.float32); b1 = np.asarray(b1, np.float32)
    W2 = np.asarray(W2, np.float32); b2 = np.asarray(b2, np.float32)
    W3 = np.asarray(W3, np.float32); b3 = np.asarray(b3, np.float32)
    wb = np.zeros(N_WB, np.float32)
    b2a = b2 - W2.sum(axis=1)            # absorb elu(+1) shift
    b3a = b3 - W3.sum(axis=1)
    wb[0] = b3a[0]

    eye = np.eye(P, dtype=np.float32)
    wdf = np.empty((N_WD, P, P), np.float32)
    wdf[_iWD_I()] = eye
    for k in range(2):
        for j in range(3):
            wdf[_iWD_W1(k, j)] = eye * W1[k, j]
        wdf[_iWD_B1(k)] = eye * b1[k]
    for m in range(2):
        for j in range(2):
            wdf[_iWD_W2(m, j)] = eye * W2[m, j]
        wdf[_iWD_B2(m)] = eye * b2a[m]
    for j in range(2):
        wdf[_iWD_W3(j)] = eye * W3[0, j]
    return wb, wdf.astype(np.float16)


def _pack_x(x2d):
    """[n*TILE, 9] float -> [n_tiles*P, 9*T] fp16, channel-major per tile."""
    n = x2d.shape[0] // TILE
    xt = x2d.reshape(n, P, T, 9).transpose(0, 1, 3, 2)
    return np.ascontiguousarray(xt, dtype=np.float16).reshape(n * P, 9 * T)


LAST_RESULTS = None  # BassKernelResults of the most recent run (for test.py)


def kernel(X, W1, b1, W2, b2, W3, b3):
    from concourse.bass_utils import run_bass_kernel_spmd
    global LAST_RESULTS

    X = np.asarray(X, np.float32).reshape(B, 9)
    wb, wd = _pack_weights(W1, b1, W2, b2, W3, b3)

    key = (COMPUTE_DT, 1)
    if key not in _CACHE:
        _CACHE[key] = _build(COMPUTE_DT)
    nc = _CACHE[key]

    in_maps = [
        {"X": _pack_x(X[c * B_CORE:(c + 1) * B_CORE]),
         "WB": wb, "WD": wd}
        for c in range(N_CORES)
    ]
    res = run_bass_kernel_spmd(nc, in_maps, core_ids=list(range(N_CORES)))
    LAST_RESULTS = res
    out = np.concatenate(
        [res.results[c]["Y"].astype(np.float32).reshape(B_CORE)
         for c in range(N_CORES)], axis=0)
    return out.reshape(B, 1)


# revision 21
# speedup vs baseline: 1.0016x; 1.0016x over previous
# Trainium2 Bass kernel for nn_Net_4861902979707
#
# Computation (per sample, B = 4194304):
#   X [B, 3, 3] -> 3 pairwise Euclidean distances d = [d01, d02, d12]
#   h1 = elu(d @ W1.T + b1); h2 = elu(h1 @ W2.T + b2); y = h2 @ W3.T + b3
#
# Strategy: pure data parallel over 8 NeuronCores (batch split). Host does
# layout/dtype only: X is cast to fp16 and each 128xT tile is transposed to
# channel-major [128 partitions, 9 coord planes, T samples] so every on-chip
# op is a fat contiguous instruction.
#
# The per-tile computation is a long cross-engine chain; engines execute
# their streams strictly in order, so naive tile-by-tile emission runs at
# the SUM of engine times (the 140us baseline). This kernel emits a 6-deep
# software pipeline with work spread across all five engines + the DMA
# CCE ALU, so at each step every engine's next instruction depends only on
# previous-step work (or an op emitted earlier in the same step):
#   step i   : DMA in tile i
#   step i+1 : DVE diffs (3 fat subs) + squares planes 0-3;
#              GPSIMD squares planes 4-8
#   step i+2 : DMA-accumulate q12 = sq6+sq7+sq8 (SWDGE accum_op=add);
#              PE q01/q02 (identity diag, PSUM); ACT sqrt(q01,q02)
#   step i+3 : ACT sqrt(q12); PE L1 (bias via ones-plane + diag passes);
#              ACT exp1/relu1 (fat); DVE h1 = min(e1,1) + r1
#   step i+4 : PE L2; ACT exp2 + relu2[k=0]; DVE relu2[k=1]
#   step i+5 : DVE m2 = min(e2,1); PE L3 (split-feed r2/m2 + bias);
#              ACT final copy -> fp16; DMA out
# ELU identity: elu(z)+1 = relu(z) + min(exp(z), 1); the +1 shift is
# absorbed into the next layer's bias on the host (b' = b - W @ 1).
# Everything on chip is fp16 (PSUM fp32), rel err ~1e-3.
import os as _os
import numpy as np

B = 4194304
N_CORES = 8
B_CORE = B // N_CORES          # 524288
P = 128                        # partitions
T = int(_os.environ.get("TSZ", "512"))
TILE = P * T
N_TILES = B_CORE // TILE

SQ_DVE = int(_os.environ.get("SQ_DVE", "6"))   # square planes on DVE
RELU2K1 = _os.environ.get("RELU2K1", "act")    # dve | act
RELU1 = _os.environ.get("RELU1", "dve")        # act | dve (fat TS from PSUM)
RELU2 = _os.environ.get("RELU2", "dve")        # act | dve
FINAL = _os.environ.get("FINAL", "act")        # act | dve
Q12 = _os.environ.get("Q12", "pe")            # dma | pe
DEPTH = 6
# bench-only: multiply tile count inside one bench_small body to measure
# steady-state vs pipeline-ramp contributions
BENCH_MULT = int(_os.environ.get("BENCH_MULT", "1"))

COMPUTE_DT = "fp16"

_CACHE = {}


def _split_sync_waits(nc, mybir, limit=1):
    """This walrus build rejects instructions carrying more than ~1 sem wait
    ("Too many sync wait commands"). Hoist excess waits onto NoOp carrier
    instructions (same engine, immediately before) — engine program order
    preserves the blocking semantics."""
    n_split = 0
    for f in nc.m.functions:
        for b in f.blocks:
            lst = b.instructions
            out = []
            changed = False
            for inst in lst:
                si = inst.sync_info
                if si is not None and len(si.on_wait) > limit:
                    waits = list(si.on_wait)
                    extra, keep = waits[:-limit], waits[-limit:]
                    for wi, w in enumerate(extra):
                        nop = mybir.InstNoOp(
                            name=f"wsplit-{inst.name}-{wi}")
                        nop.engine = inst.engine
                        nop.sync_info = mybir.SyncInfo(
                            on_wait=[w], on_update=[])
                        out.append(nop)
                        n_split += 1
                    inst.sync_info = type(si)(
                        on_wait=keep, on_update=list(si.on_update))
                    changed = True
                out.append(inst)
            if changed:
                b.instructions = out
    return n_split


# WD diag-matrix indices (each a [128,128] fp16 lhsT)
def _iWD_I():
    return 0
def _iWD_W1(k, j):
    return 1 + 3 * k + j
def _iWD_W2(m, j):
    return 7 + 2 * m + j
def _iWD_B1(k):
    return 11 + k
def _iWD_B2(m):
    return 13 + m
def _iWD_W3(j):
    return 15 + j
N_WD = 17
N_WB = 1  # slot 0: b3a (final-activation bias)


def _build(dt_name=None, reps=1, bench_small=False):
    import concourse.bass as bass
    import concourse.tile as tile
    import concourse.mybir as mybir

    f32 = mybir.dt.float32
    f16 = mybir.dt.float16
    Alu = mybir.AluOpType
    Act = mybir.ActivationFunctionType

    nc = bass.Bass()
    NT = 1 if bench_small else N_TILES
    X = nc.dram_tensor("X", [NT * P, 9 * T], f16, kind="ExternalInput")
    WB = nc.dram_tensor("WB", [N_WB], f32, kind="ExternalInput")
    WD = nc.dram_tensor("WD", [N_WD, P, P], f16, kind="ExternalInput")
    Y = nc.dram_tensor("Y", [NT * P, T], f16, kind="ExternalOutput")

    with tile.TileContext(nc) as tc:
        with (
            tc.tile_pool(name="singles", bufs=1) as singles,
            tc.tile_pool(name="xin", bufs=4) as xin,
            tc.tile_pool(name="diffp", bufs=3) as diffp,
            tc.tile_pool(name="distp", bufs=3) as distp,
            tc.tile_pool(name="elup", bufs=3) as elup,
            tc.tile_pool(name="h1p", bufs=3) as h1p,
            tc.tile_pool(name="outp", bufs=3) as outp,
            tc.tile_pool(name="ps1", bufs=1, space="PSUM") as ps1,
        ):
            wb = singles.tile([P, N_WB], f32)
            nc.gpsimd.dma_start(
                out=wb[:],
                in_=bass.AP(tensor=WB[:].tensor, offset=0,
                            ap=[[0, P], [1, N_WB]]))
            wd = singles.tile([P, N_WD, P], f16)
            nc.sync.dma_start(
                out=wd[:],
                in_=bass.AP(tensor=WD[:].tensor, offset=0,
                            ap=[[P, P], [P * P, N_WD], [1, P]]))
            ones = singles.tile([P, T], f16)
            nc.vector.memset(ones[:], 1.0)
            ones2 = singles.tile([P, 2, T], f16)
            nc.vector.memset(ones2[:], 1.0)

            def diag(i):  # [128,128] lhsT AP
                return wd[:, i, :]

            WIN = DEPTH + 1
            st = [dict() for _ in range(WIN)]

            def S(i):
                return st[i % WIN]

            d0 = SQ_DVE  # DVE squares planes [0, d0); GP squares [d0, 9)

            NT_body = N_TILES * (BENCH_MULT if bench_small else 1)

            def step(s):
                i_dma, i_s1, i_s2 = s, s - 1, s - 2
                i_b1, i_b2, i_b3 = s - 3, s - 4, s - 5

                def live(i):
                    return 0 <= i < NT_body

                # 1. DMA in (tile s)
                if live(i_dma):
                    src = 0 if bench_small else i_dma
                    xt = xin.tile([P, 9, T], f16)
                    nc.sync.dma_start(
                        out=xt[:], in_=X[src * P:(src + 1) * P, :])
                    S(i_dma)["xt"] = xt

                # 2a. ACT sqrt(q12)  [q12 DMA-accumulated last step]
                if live(i_b1) and Q12 == "dma":
                    d = S(i_b1)
                    nc.scalar.activation(
                        d["dist"][:, 2, :], d["q12"], Act.Sqrt)

                # 2. PE L1 (bias plane + diag passes)
                if live(i_b1):
                    d = S(i_b1)
                    z1 = ps1.tile([P, 2, T], f32, tag="z1")
                    for k in range(2):
                        nc.tensor.matmul(
                            z1[:, k, :], diag(_iWD_B1(k)), ones[:],
                            start=True, stop=False)
                        for j in range(3):
                            nc.tensor.matmul(
                                z1[:, k, :], diag(_iWD_W1(k, j)),
                                d["dist"][:, j, :],
                                start=False, stop=(j == 2))
                    d["z1"] = z1

                # 4. DVE m2 = min(e2, 1)  [e2 from last step]
                if live(i_b3):
                    d = S(i_b3)
                    m2 = outp.tile([P, 2, T], f16, tag="m2")
                    nc.vector.tensor_tensor(
                        out=m2[:], in0=d["e2"][:], in1=ones2[:],
                        op=Alu.min)
                    d["m2"] = m2

                # 5-7. DVE diffs + squares; GP squares
                if live(i_s1):
                    d = S(i_s1)
                    xt = d["xt"]
                    diff = diffp.tile([P, 9, T], f16)
                    nc.vector.tensor_sub(
                        diff[:, 6:9, :], xt[:, 3:6, :], xt[:, 6:9, :])
                    nc.gpsimd.tensor_mul(
                        diff[:, 6:9, :], diff[:, 6:9, :], diff[:, 6:9, :])
                    nc.vector.tensor_sub(
                        diff[:, 0:3, :], xt[:, 0:3, :], xt[:, 3:6, :])
                    nc.vector.tensor_sub(
                        diff[:, 3:6, :], xt[:, 0:3, :], xt[:, 6:9, :])
                    nc.vector.tensor_mul(
                        diff[:, 0:6, :], diff[:, 0:6, :], diff[:, 0:6, :])
                    d["diff"] = diff

                # 9-10. ACT exp1 / relu1 (fat, bias already in PSUM)
                if live(i_b1):
                    d = S(i_b1)
                    e1 = elup.tile([P, 2, T], f16, tag="e1")
                    r1 = elup.tile([P, 2, T], f16, tag="r1")
                    nc.scalar.activation(e1[:], d["z1"][:], Act.Exp)
                    if RELU1 == "dve":
                        nc.vector.tensor_scalar(
                            out=r1[:], in0=d["z1"][:], scalar1=0.0,
                            scalar2=None, op0=Alu.max)
                    else:
                        nc.scalar.activation(r1[:], d["z1"][:], Act.Relu)
                    d["e1"], d["r1"] = e1, r1

                # 11. DVE h1 = min(e1,1) + r1
                if live(i_b1):
                    d = S(i_b1)
                    h1 = h1p.tile([P, 2, T], f16, tag="h1")
                    nc.vector.tensor_tensor(
                        out=h1[:], in0=d["e1"][:], in1=ones2[:], op=Alu.min)
                    nc.vector.tensor_add(h1[:], h1[:], d["r1"][:])
                    d["h1"] = h1

                # 12. PE q01/q02 (+ q12 if not on DMA)
                if live(i_s2):
                    d = S(i_s2)
                    diff = d["diff"]
                    if Q12 == "dma":
                        q12 = distp.tile([P, T], f16, tag="q12")
                        nc.vector.memset(q12[:], 0.0)
                        q12rep = bass.AP(
                            tensor=q12[:].tensor, offset=q12[:].offset,
                            ap=[q12[:].ap[0], [0, 3], [1, T]])
                        nc.gpsimd.dma_start(
                            out=q12rep, in_=diff[:, 6:9, :],
                            accum_op=Alu.add)
                        d["q12"] = q12
                    npair = 2 if Q12 == "dma" else 3
                    qp = ps1.tile([P, npair, T], f32, tag="q")
                    for pi in range(npair):
                        for c in range(3):
                            nc.tensor.matmul(
                                qp[:, pi, :], diag(_iWD_I()),
                                diff[:, 3 * pi + c, :],
                                start=(c == 0), stop=(c == 2))
                    d["qp"] = qp

                # 13. PE L2 (bias + diag passes over h1)
                if live(i_b2):
                    d = S(i_b2)
                    z2 = ps1.tile([P, 2, T], f32, tag="z2")
                    for m_ in range(2):
                        nc.tensor.matmul(
                            z2[:, m_, :], diag(_iWD_B2(m_)), ones[:],
                            start=True, stop=False)
                        for j in range(2):
                            nc.tensor.matmul(
                                z2[:, m_, :], diag(_iWD_W2(m_, j)),
                                d["h1"][:, j, :],
                                start=False, stop=(j == 1))
                    d["z2"] = z2

                # 14. PE L3 (split feed: bias + w3*(r2_j, m2_j))
                if live(i_b3):
                    d = S(i_b3)
                    yz = ps1.tile([P, T], f32, tag="yz")
                    for j in range(2):
                        nc.tensor.matmul(
                            yz[:], diag(_iWD_W3(j)), d["r2"][:, j, :],
                            start=(j == 0), stop=False)
                    for j in range(2):
                        nc.tensor.matmul(
                            yz[:], diag(_iWD_W3(j)), d["m2"][:, j, :],
                            start=False, stop=(j == 1))
                    d["yz"] = yz

                # 15. ACT sqrt (pairs summed in PSUM, fat)
                if live(i_s2):
                    d = S(i_s2)
                    dist = distp.tile([P, 3, T], f16, tag="dist")
                    if Q12 == "dma":
                        nc.scalar.activation(
                            dist[:, 0:2, :], d["qp"][:], Act.Sqrt)
                    else:
                        nc.scalar.activation(
                            dist[:, :, :], d["qp"][:], Act.Sqrt)
                    d["dist"] = dist

                # 16-18. elu2: ACT exp2 (fat) + relu2 split ACT/DVE
                if live(i_b2):
                    d = S(i_b2)
                    e2 = elup.tile([P, 2, T], f16, tag="e2")
                    r2 = elup.tile([P, 2, T], f16, tag="r2")
                    nc.scalar.activation(e2[:], d["z2"][:], Act.Exp)
                    if RELU2 == "dve":
                        nc.vector.tensor_scalar(
                            out=r2[:], in0=d["z2"][:], scalar1=0.0,
                            scalar2=None, op0=Alu.max)
                    elif RELU2K1 == "dve":
                        nc.scalar.activation(
                            r2[:, 0, :], d["z2"][:, 0, :], Act.Relu)
                        nc.vector.tensor_scalar(
                            out=r2[:, 1, :], in0=d["z2"][:, 1, :],
                            scalar1=0.0, scalar2=None, op0=Alu.max)
                    else:
                        nc.scalar.activation(r2[:], d["z2"][:], Act.Relu)
                    
                    d["e2"], d["r2"] = e2, r2

                # 19. final: add b3a, downcast -> fp16
                if live(i_b3):
                    d = S(i_b3)
                    yt = outp.tile([P, T], f16, tag="yt")
                    if FINAL == "dve":
                        nc.vector.tensor_scalar(
                            out=yt[:], in0=d["yz"][:], scalar1=wb[:, 0:1],
                            scalar2=None, op0=Alu.add)
                    else:
                        nc.scalar.activation(
                            yt[:], d["yz"][:], Act.Identity,
                            bias=wb[:, 0:1], scale=1.0)
                    d["yt"] = yt

                # 20. DMA out
                if live(i_b3):
                    src = 0 if bench_small else i_b3
                    nc.sync.dma_start(
                        out=Y[src * P:(src + 1) * P, :], in_=S(i_b3)["yt"][:])

            _loop = tc.For_i(0, reps) if reps != 1 else None
            if _loop is not None:
                _loop.__enter__()

            for s in range(NT_body + DEPTH - 1):
                step(s)

            if _loop is not None:
                _loop.__exit__(None, None, None)

    _split_sync_waits(nc, mybir, limit=1)
    return nc


def _pack_weights(W1, b1, W2, b2, W3, b3):
    W1 = np.asarray(W1, np.float32); b1 = np.asarray(b1, np.float32)
    W2 = np.asarray(W2, np.float32); b2 = np.asarray(b2, np.float32)
    W3 = np.asarray(W3, np.float32); b3 = np.asarray(b3, np.float32)
    wb = np.zeros(N_WB, np.float32)
    b2a = b2 - W2.sum(axis=1)            # absorb elu(+1) shift
    b3a = b3 - W3.sum(axis=1)
    wb[0] = b3a[0]

    eye = np.eye(P, dtype=np.float32)
    wdf = np.empty((N_WD, P, P), np.float32)
    wdf[_iWD_I()] = eye
    for k in range(2):
        for j in range(3):
            wdf[_iWD_W1(k, j)] = eye * W1[k, j]
        wdf[_iWD_B1(k)] = eye * b1[k]
    for m in range(2):
        for j in range(2):
            wdf[_iWD_W2(m, j)] = eye * W2[m, j]
        wdf[_iWD_B2(m)] = eye * b2a[m]
    for j in range(2):
        wdf[_iWD_W3(j)] = eye * W3[0, j]
    return wb, wdf.astype(np.float16)


def _pack_x(x2d):
    """[n*TILE, 9] float -> [n_tiles*P, 9*T] fp16, channel-major per tile."""
    n = x2d.shape[0] // TILE
    xt = x2d.reshape(n, P, T, 9).transpose(0, 1, 3, 2)
    return np.ascontiguousarray(xt, dtype=np.float16).reshape(n * P, 9 * T)


LAST_RESULTS = None  # BassKernelResults of the most recent run (for test.py)


def kernel(X, W1, b1, W2, b2, W3, b3):
    from concourse.bass_utils import run_bass_kernel_spmd
    global LAST_RESULTS

    X = np.asarray(X, np.float32).reshape(B, 9)
    wb, wd = _pack_weights(W1, b1, W2, b2, W3, b3)

    key = (COMPUTE_DT, 1)
    if key not in _CACHE:
        _CACHE[key] = _build(COMPUTE_DT)
    nc = _CACHE[key]

    in_maps = [
        {"X": _pack_x(X[c * B_CORE:(c + 1) * B_CORE]),
         "WB": wb, "WD": wd}
        for c in range(N_CORES)
    ]
    res = run_bass_kernel_spmd(nc, in_maps, core_ids=list(range(N_CORES)))
    LAST_RESULTS = res
    out = np.concatenate(
        [res.results[c]["Y"].astype(np.float32).reshape(B_CORE)
         for c in range(N_CORES)], axis=0)
    return out.reshape(B, 1)


# revision 22
# speedup vs baseline: 1.0526x; 1.0509x over previous
# Trainium2 Bass kernel for nn_Net_4861902979707
#
# Computation (per sample, B = 4194304):
#   X [B, 3, 3] -> 3 pairwise Euclidean distances d = [d01, d02, d12]
#   h1 = elu(d @ W1.T + b1); h2 = elu(h1 @ W2.T + b2); y = h2 @ W3.T + b3
#
# Strategy: pure data parallel over 8 NeuronCores (batch split). Host does
# layout/dtype only: X is cast to fp16 and each 128xT tile is transposed to
# channel-major [128 partitions, 9 coord planes, T samples] so every on-chip
# op is a fat contiguous instruction.
#
# The per-tile computation is a long cross-engine chain; engines execute
# their streams strictly in order, so naive tile-by-tile emission runs at
# the SUM of engine times (the 140us baseline). This kernel emits a 6-deep
# software pipeline with work spread across all five engines + the DMA
# CCE ALU, so at each step every engine's next instruction depends only on
# previous-step work (or an op emitted earlier in the same step):
#   step i   : DMA in tile i
#   step i+1 : DVE diffs (3 fat subs) + squares planes 0-3;
#              GPSIMD squares planes 4-8
#   step i+2 : DMA-accumulate q12 = sq6+sq7+sq8 (SWDGE accum_op=add);
#              PE q01/q02 (identity diag, PSUM); ACT sqrt(q01,q02)
#   step i+3 : ACT sqrt(q12); PE L1 (bias via ones-plane + diag passes);
#              ACT exp1/relu1 (fat); DVE h1 = min(e1,1) + r1
#   step i+4 : PE L2; ACT exp2 + relu2[k=0]; DVE relu2[k=1]
#   step i+5 : DVE m2 = min(e2,1); PE L3 (split-feed r2/m2 + bias);
#              ACT final copy -> fp16; DMA out
# ELU identity: elu(z)+1 = relu(z) + min(exp(z), 1); the +1 shift is
# absorbed into the next layer's bias on the host (b' = b - W @ 1).
# Everything on chip is fp16 (PSUM fp32), rel err ~1e-3.
import os as _os
import numpy as np

B = 4194304
N_CORES = 8
B_CORE = B // N_CORES          # 524288
P = 128                        # partitions
T = int(_os.environ.get("TSZ", "512"))
TILE = P * T
N_TILES = B_CORE // TILE

SQ_DVE = int(_os.environ.get("SQ_DVE", "6"))   # square planes on DVE
RELU2K1 = _os.environ.get("RELU2K1", "act")    # dve | act
RELU1 = _os.environ.get("RELU1", "dve")        # act | dve (fat TS from PSUM)
RELU2 = _os.environ.get("RELU2", "dve")        # act | dve
FINAL = _os.environ.get("FINAL", "act")        # act | dve
H1 = _os.environ.get("H1", "stt")              # stt (fused) | tt (min+add)
Q12 = _os.environ.get("Q12", "pe")            # dma | pe
DEPTH = 6
# bench-only: multiply tile count inside one bench_small body to measure
# steady-state vs pipeline-ramp contributions
BENCH_MULT = int(_os.environ.get("BENCH_MULT", "1"))

COMPUTE_DT = "fp16"

_CACHE = {}


def _split_sync_waits(nc, mybir, limit=1):
    """This walrus build rejects instructions carrying more than ~1 sem wait
    ("Too many sync wait commands"). Hoist excess waits onto NoOp carrier
    instructions (same engine, immediately before) — engine program order
    preserves the blocking semantics."""
    n_split = 0
    for f in nc.m.functions:
        for b in f.blocks:
            lst = b.instructions
            out = []
            changed = False
            for inst in lst:
                si = inst.sync_info
                if si is not None and len(si.on_wait) > limit:
                    waits = list(si.on_wait)
                    extra, keep = waits[:-limit], waits[-limit:]
                    for wi, w in enumerate(extra):
                        nop = mybir.InstNoOp(
                            name=f"wsplit-{inst.name}-{wi}")
                        nop.engine = inst.engine
                        nop.sync_info = mybir.SyncInfo(
                            on_wait=[w], on_update=[])
                        out.append(nop)
                        n_split += 1
                    inst.sync_info = type(si)(
                        on_wait=keep, on_update=list(si.on_update))
                    changed = True
                out.append(inst)
            if changed:
                b.instructions = out
    return n_split


# WD diag-matrix indices (each a [128,128] fp16 lhsT)
def _iWD_I():
    return 0
def _iWD_W1(k, j):
    return 1 + 3 * k + j
def _iWD_W2(m, j):
    return 7 + 2 * m + j
def _iWD_B1(k):
    return 11 + k
def _iWD_B2(m):
    return 13 + m
def _iWD_W3(j):
    return 15 + j
N_WD = 17
N_WB = 1  # slot 0: b3a (final-activation bias)


def _build(dt_name=None, reps=1, bench_small=False):
    import concourse.bass as bass
    import concourse.tile as tile
    import concourse.mybir as mybir

    f32 = mybir.dt.float32
    f16 = mybir.dt.float16
    Alu = mybir.AluOpType
    Act = mybir.ActivationFunctionType

    nc = bass.Bass()
    NT = 1 if bench_small else N_TILES
    X = nc.dram_tensor("X", [NT * P, 9 * T], f16, kind="ExternalInput")
    WB = nc.dram_tensor("WB", [N_WB], f32, kind="ExternalInput")
    WD = nc.dram_tensor("WD", [N_WD, P, P], f16, kind="ExternalInput")
    Y = nc.dram_tensor("Y", [NT * P, T], f16, kind="ExternalOutput")

    with tile.TileContext(nc) as tc:
        with (
            tc.tile_pool(name="singles", bufs=1) as singles,
            tc.tile_pool(name="xin", bufs=4) as xin,
            tc.tile_pool(name="diffp", bufs=3) as diffp,
            tc.tile_pool(name="distp", bufs=3) as distp,
            tc.tile_pool(name="elup", bufs=3) as elup,
            tc.tile_pool(name="h1p", bufs=3) as h1p,
            tc.tile_pool(name="outp", bufs=3) as outp,
            tc.tile_pool(name="ps1", bufs=1, space="PSUM") as ps1,
        ):
            wb = singles.tile([P, N_WB], f32)
            nc.gpsimd.dma_start(
                out=wb[:],
                in_=bass.AP(tensor=WB[:].tensor, offset=0,
                            ap=[[0, P], [1, N_WB]]))
            wd = singles.tile([P, N_WD, P], f16)
            nc.sync.dma_start(
                out=wd[:],
                in_=bass.AP(tensor=WD[:].tensor, offset=0,
                            ap=[[P, P], [P * P, N_WD], [1, P]]))
            ones = singles.tile([P, T], f16)
            nc.vector.memset(ones[:], 1.0)
            ones2 = singles.tile([P, 2, T], f16)
            nc.vector.memset(ones2[:], 1.0)

            def diag(i):  # [128,128] lhsT AP
                return wd[:, i, :]

            WIN = DEPTH + 1
            st = [dict() for _ in range(WIN)]

            def S(i):
                return st[i % WIN]

            d0 = SQ_DVE  # DVE squares planes [0, d0); GP squares [d0, 9)

            NT_body = N_TILES * (BENCH_MULT if bench_small else 1)

            def step(s):
                i_dma, i_s1, i_s2 = s, s - 1, s - 2
                i_b1, i_b2, i_b3 = s - 3, s - 4, s - 5

                def live(i):
                    return 0 <= i < NT_body

                # 1. DMA in (tile s)
                if live(i_dma):
                    src = 0 if bench_small else i_dma
                    xt = xin.tile([P, 9, T], f16)
                    nc.sync.dma_start(
                        out=xt[:], in_=X[src * P:(src + 1) * P, :])
                    S(i_dma)["xt"] = xt

                # 2a. ACT sqrt(q12)  [q12 DMA-accumulated last step]
                if live(i_b1) and Q12 == "dma":
                    d = S(i_b1)
                    nc.scalar.activation(
                        d["dist"][:, 2, :], d["q12"], Act.Sqrt)

                # 2. PE L1 (bias plane + diag passes)
                if live(i_b1):
                    d = S(i_b1)
                    z1 = ps1.tile([P, 2, T], f32, tag="z1")
                    for k in range(2):
                        nc.tensor.matmul(
                            z1[:, k, :], diag(_iWD_B1(k)), ones[:],
                            start=True, stop=False)
                        for j in range(3):
                            nc.tensor.matmul(
                                z1[:, k, :], diag(_iWD_W1(k, j)),
                                d["dist"][:, j, :],
                                start=False, stop=(j == 2))
                    d["z1"] = z1

                # 4. DVE m2 = min(e2, 1)  [e2 from last step]
                if live(i_b3):
                    d = S(i_b3)
                    m2 = outp.tile([P, 2, T], f16, tag="m2")
                    nc.vector.tensor_tensor(
                        out=m2[:], in0=d["e2"][:], in1=ones2[:],
                        op=Alu.min)
                    d["m2"] = m2

                # 5-7. DVE diffs + squares; GP squares
                if live(i_s1):
                    d = S(i_s1)
                    xt = d["xt"]
                    diff = diffp.tile([P, 9, T], f16)
                    nc.vector.tensor_sub(
                        diff[:, 6:9, :], xt[:, 3:6, :], xt[:, 6:9, :])
                    nc.gpsimd.tensor_mul(
                        diff[:, 6:9, :], diff[:, 6:9, :], diff[:, 6:9, :])
                    nc.vector.tensor_sub(
                        diff[:, 0:3, :], xt[:, 0:3, :], xt[:, 3:6, :])
                    nc.vector.tensor_sub(
                        diff[:, 3:6, :], xt[:, 0:3, :], xt[:, 6:9, :])
                    nc.vector.tensor_mul(
                        diff[:, 0:6, :], diff[:, 0:6, :], diff[:, 0:6, :])
                    d["diff"] = diff

                # 9-10. ACT exp1 / relu1 (fat, bias already in PSUM)
                if live(i_b1):
                    d = S(i_b1)
                    e1 = elup.tile([P, 2, T], f16, tag="e1")
                    r1 = elup.tile([P, 2, T], f16, tag="r1")
                    nc.scalar.activation(e1[:], d["z1"][:], Act.Exp)
                    if RELU1 == "dve":
                        nc.vector.tensor_scalar(
                            out=r1[:], in0=d["z1"][:], scalar1=0.0,
                            scalar2=None, op0=Alu.max)
                    else:
                        nc.scalar.activation(r1[:], d["z1"][:], Act.Relu)
                    d["e1"], d["r1"] = e1, r1

                # 11. DVE h1 = min(e1,1) + r1
                if live(i_b1):
                    d = S(i_b1)
                    h1 = h1p.tile([P, 2, T], f16, tag="h1")
                    if H1 == "stt":
                        nc.vector.scalar_tensor_tensor(
                            out=h1[:], in0=d["e1"][:], scalar=1.0,
                            in1=d["r1"][:], op0=Alu.min, op1=Alu.add)
                    else:
                        nc.vector.tensor_tensor(
                            out=h1[:], in0=d["e1"][:], in1=ones2[:],
                            op=Alu.min)
                        nc.vector.tensor_add(h1[:], h1[:], d["r1"][:])
                    d["h1"] = h1

                # 12. PE q01/q02 (+ q12 if not on DMA)
                if live(i_s2):
                    d = S(i_s2)
                    diff = d["diff"]
                    if Q12 == "dma":
                        q12 = distp.tile([P, T], f16, tag="q12")
                        nc.vector.memset(q12[:], 0.0)
                        q12rep = bass.AP(
                            tensor=q12[:].tensor, offset=q12[:].offset,
                            ap=[q12[:].ap[0], [0, 3], [1, T]])
                        nc.gpsimd.dma_start(
                            out=q12rep, in_=diff[:, 6:9, :],
                            accum_op=Alu.add)
                        d["q12"] = q12
                    npair = 2 if Q12 == "dma" else 3
                    qp = ps1.tile([P, npair, T], f32, tag="q")
                    for pi in range(npair):
                        for c in range(3):
                            nc.tensor.matmul(
                                qp[:, pi, :], diag(_iWD_I()),
                                diff[:, 3 * pi + c, :],
                                start=(c == 0), stop=(c == 2))
                    d["qp"] = qp

                # 13. PE L2 (bias + diag passes over h1)
                if live(i_b2):
                    d = S(i_b2)
                    z2 = ps1.tile([P, 2, T], f32, tag="z2")
                    for m_ in range(2):
                        nc.tensor.matmul(
                            z2[:, m_, :], diag(_iWD_B2(m_)), ones[:],
                            start=True, stop=False)
                        for j in range(2):
                            nc.tensor.matmul(
                                z2[:, m_, :], diag(_iWD_W2(m_, j)),
                                d["h1"][:, j, :],
                                start=False, stop=(j == 1))
                    d["z2"] = z2

                # 14. PE L3 (split feed: bias + w3*(r2_j, m2_j))
                if live(i_b3):
                    d = S(i_b3)
                    yz = ps1.tile([P, T], f32, tag="yz")
                    for j in range(2):
                        nc.tensor.matmul(
                            yz[:], diag(_iWD_W3(j)), d["r2"][:, j, :],
                            start=(j == 0), stop=False)
                    for j in range(2):
                        nc.tensor.matmul(
                            yz[:], diag(_iWD_W3(j)), d["m2"][:, j, :],
                            start=False, stop=(j == 1))
                    d["yz"] = yz

                # 15. ACT sqrt (pairs summed in PSUM, fat)
                if live(i_s2):
                    d = S(i_s2)
                    dist = distp.tile([P, 3, T], f16, tag="dist")
                    if Q12 == "dma":
                        nc.scalar.activation(
                            dist[:, 0:2, :], d["qp"][:], Act.Sqrt)
                    else:
                        nc.scalar.activation(
                            dist[:, :, :], d["qp"][:], Act.Sqrt)
                    d["dist"] = dist

                # 16-18. elu2: ACT exp2 (fat) + relu2 split ACT/DVE
                if live(i_b2):
                    d = S(i_b2)
                    e2 = elup.tile([P, 2, T], f16, tag="e2")
                    r2 = elup.tile([P, 2, T], f16, tag="r2")
                    nc.scalar.activation(e2[:], d["z2"][:], Act.Exp)
                    if RELU2 == "dve":
                        nc.vector.tensor_scalar(
                            out=r2[:], in0=d["z2"][:], scalar1=0.0,
                            scalar2=None, op0=Alu.max)
                    elif RELU2K1 == "dve":
                        nc.scalar.activation(
                            r2[:, 0, :], d["z2"][:, 0, :], Act.Relu)
                        nc.vector.tensor_scalar(
                            out=r2[:, 1, :], in0=d["z2"][:, 1, :],
                            scalar1=0.0, scalar2=None, op0=Alu.max)
                    else:
                        nc.scalar.activation(r2[:], d["z2"][:], Act.Relu)
                    
                    d["e2"], d["r2"] = e2, r2

                # 19. final: add b3a, downcast -> fp16
                if live(i_b3):
                    d = S(i_b3)
                    yt = outp.tile([P, T], f16, tag="yt")
                    if FINAL == "dve":
                        nc.vector.tensor_scalar(
                            out=yt[:], in0=d["yz"][:], scalar1=wb[:, 0:1],
                            scalar2=None, op0=Alu.add)
                    else:
                        nc.scalar.activation(
                            yt[:], d["yz"][:], Act.Identity,
                            bias=wb[:, 0:1], scale=1.0)
                    d["yt"] = yt

                # 20. DMA out
                if live(i_b3):
                    src = 0 if bench_small else i_b3
                    nc.sync.dma_start(
                        out=Y[src * P:(src + 1) * P, :], in_=S(i_b3)["yt"][:])

            _loop = tc.For_i(0, reps) if reps != 1 else None
            if _loop is not None:
                _loop.__enter__()

            for s in range(NT_body + DEPTH - 1):
                step(s)

            if _loop is not None:
                _loop.__exit__(None, None, None)

    _split_sync_waits(nc, mybir, limit=1)
    return nc


def _pack_weights(W1, b1, W2, b2, W3, b3):
    W1 = np.asarray(W1, np.float32); b1 = np.asarray(b1, np.float32)
    W2 = np.asarray(W2, np.float32); b2 = np.asarray(b2, np.float32)
    W3 = np.asarray(W3, np.float32); b3 = np.asarray(b3, np.float32)
    wb = np.zeros(N_WB, np.float32)
    b2a = b2 - W2.sum(axis=1)            # absorb elu(+1) shift
    b3a = b3 - W3.sum(axis=1)
    wb[0] = b3a[0]

    eye = np.eye(P, dtype=np.float32)
    wdf = np.empty((N_WD, P, P), np.float32)
    wdf[_iWD_I()] = eye
    for k in range(2):
        for j in range(3):
            wdf[_iWD_W1(k, j)] = eye * W1[k, j]
        wdf[_iWD_B1(k)] = eye * b1[k]
    for m in range(2):
        for j in range(2):
            wdf[_iWD_W2(m, j)] = eye * W2[m, j]
        wdf[_iWD_B2(m)] = eye * b2a[m]
    for j in range(2):
        wdf[_iWD_W3(j)] = eye * W3[0, j]
    return wb, wdf.astype(np.float16)


def _pack_x(x2d):
    """[n*TILE, 9] float -> [n_tiles*P, 9*T] fp16, channel-major per tile."""
    n = x2d.shape[0] // TILE
    xt = x2d.reshape(n, P, T, 9).transpose(0, 1, 3, 2)
    return np.ascontiguousarray(xt, dtype=np.float16).reshape(n * P, 9 * T)


LAST_RESULTS = None  # BassKernelResults of the most recent run (for test.py)


def kernel(X, W1, b1, W2, b2, W3, b3):
    from concourse.bass_utils import run_bass_kernel_spmd
    global LAST_RESULTS

    X = np.asarray(X, np.float32).reshape(B, 9)
    wb, wd = _pack_weights(W1, b1, W2, b2, W3, b3)

    key = (COMPUTE_DT, 1)
    if key not in _CACHE:
        _CACHE[key] = _build(COMPUTE_DT)
    nc = _CACHE[key]

    in_maps = [
        {"X": _pack_x(X[c * B_CORE:(c + 1) * B_CORE]),
         "WB": wb, "WD": wd}
        for c in range(N_CORES)
    ]
    res = run_bass_kernel_spmd(nc, in_maps, core_ids=list(range(N_CORES)))
    LAST_RESULTS = res
    out = np.concatenate(
        [res.results[c]["Y"].astype(np.float32).reshape(B_CORE)
         for c in range(N_CORES)], axis=0)
    return out.reshape(B, 1)
